# revision 12
# baseline (speedup 1.0000x reference)
"""Trainium2 Bass kernel for the vq_codebook classifier problem.

Computes, for X [4096, 512] f32 and grp [1, 512, 100] f32:
    l1   = sum_d |X[n,d] - grp[0,d,c]|             -> [N, C]
    norm = softmax(-l1, axis=1)
    cs   = (X @ g) / max(|X| * |g|, eps)           (cosine similarity)
    out  = max_c(cs) * softmax(cs, axis=1) * norm

Sharding: data-parallel over N across 8 NeuronCores (512 rows each),
grp replicated.

This environment pays a large fixed cost PER INSTRUCTION (measured
~30-50us each, regardless of the work an instruction does), so the
kernel is built around a minimal instruction count with huge access
patterns:

  * All dense inputs (X row-tiled, X^T, g d-tiled, g^T, an f32
    identity) are packed host-side into ONE DRAM tensor in the exact
    SBUF layout and loaded by ONE DMA; views (incl. an f32 bitcast for
    the identity) carve it up.  Host-side packing/transposition is
    layout-only prep, part of the sharding contract; all real math
    (norms, distances, GEMM, softmaxes) runs on device.
  * The L1-distance tensor |x[n,d] - g[d,c]| for a block of 25 classes
    x all 512 rows is ONE vector-engine tensor_tensor subtract over a
    [128, 4, 25, 512] access pattern (x broadcast over the class dim
    with a stride-0 AP; the c-major prototype table - replicated to all
    128 partitions by a partition-broadcast DMA straight from DRAM -
    broadcast over the row-tile dim).  The d-reduction with |.| is ONE
    tensor_reduce(axis=X, apply_absolute_value=True).  12 instructions
    for the whole 26M-element L1 computation.
  * The cosine GEMM runs transposed ([c, n] = g16-stationary x
    X^T-moving, 4 matmuls) and is transposed back to [n, c] by 4 PE
    transposes; 1/|g| is folded in before the transpose, 1/|x| after,
    each as one big broadcasted tensor_tensor.
  * The epilogue is fused across all 4 row-tiles with [128, 4, 100]
    APs.  The softmin shift uses a per-partition (XY) min so it rides
    the activation bias port (softmax ratios are invariant to any
    per-row constant, and each row lives on one partition, so a
    per-partition constant is exact); e1/e2 share one [128, 8, 100]
    tile so one reduce yields both softmax denominators; both sqrt and
    both reciprocal calls are packed into single [128, 5] ops.
"""

import numpy as np

P = 128
R = 512          # rows per core (4096 / 8 cores)
D = 512
C = 100
RT = R // P      # 4 row-tiles
NT = D // P      # 4 d-tiles
CB = 25          # classes per L1 block
NCB = C // CB    # 4 class blocks
N_CORES = 8

# packed input layout (f16 columns per partition)
_OX = 0                    # x16   [128, 4, 512]
_OXT = _OX + RT * D        # xt16  [128, 4, 512]
_OG = _OXT + NT * R        # g16   [128, 4, 100]
_OGT = _OG + NT * C        # gt16  [100, 512] (partitions 0..99)
_OID = _OGT + D            # ident [128, 128] f32 (bitcast, 256 f16 cols)
_IN_W = _OID + 2 * P

_CACHE = {}


_SELF_SEM = {
    "EngineType.DVE": "DVE_",
    "EngineType.Activation": "Activation_",
    "EngineType.PE": "PE_",
    "EngineType.SP": "SP_",
    "EngineType.Pool": "Pool_",
}


def _split_excess_waits(nc, limit=1):
    """walrus in this container rejects instructions carrying more than
    one sync wait ("Too many sync wait commands").

    First drop waits on the instruction's OWN engine semaphore: these
    engines execute and complete their queues strictly in order (the
    only documented reorder is the PE LDWEIGHTS pull-ahead, so waits on
    InstLdweights are kept), which makes a same-engine wait redundant -
    FIFO order already guarantees it.  Then hoist any still-excess
    waits onto same-engine NoOps inserted immediately before the
    instruction."""
    import concourse.mybir as mb
    import bass_rust

    n_id = [0]

    def mknop(engine, waits):
        n_id[0] += 1
        return bass_rust.InstNoOp(
            name=f"waitsplit-{n_id[0]}", engine=engine, ins=[], outs=[],
            sync_info=mb.SyncInfo(on_wait=list(waits), on_update=[]),
        )

    for fn in nc.m.functions:
        for bb in fn.blocks:
            insts = bb.instructions
            out = []
            for inst in insts:
                si = inst.sync_info
                if si is not None and si.on_wait:
                    # NOTE: dropping waits on the instruction's own engine
                    # semaphore looks FIFO-redundant but races on hardware
                    # (measured garbage outputs) - do not remove waits.
                    waits = list(si.on_wait)
                    if len(waits) > limit:
                        extra, keep = waits[:-limit], waits[-limit:]
                        for w in extra:
                            out.append(mknop(inst.engine, [w]))
                        inst.sync_info = mb.SyncInfo(
                            on_wait=keep, on_update=list(si.on_update)
                        )
                out.append(inst)
            insts[:] = out
    return nc


def _build_nc(reps: int = 1):
    import concourse.bass as bass
    import concourse.mybir as mybir
    import concourse.tile as tile
    from contextlib import ExitStack

    f32 = mybir.dt.float32
    f16 = mybir.dt.float16
    Alu = mybir.AluOpType
    Act = mybir.ActivationFunctionType
    Ax = mybir.AxisListType

    nc = bass.Bass(target_bir_lowering=False)
    INd = nc.declare_dram_parameter("IN16", [P, _IN_W], f16, isOutput=False)
    GR16d = nc.declare_dram_parameter("GR16", [1, C * D], f16, isOutput=False)
    Yd = nc.declare_dram_parameter("Y", [R, C], f32, isOutput=True)

    with ExitStack() as ctx:
        tc = ctx.enter_context(tile.TileContext(nc))
        inp = ctx.enter_context(tc.tile_pool(name="inp", bufs=2))
        grep_pool = ctx.enter_context(tc.tile_pool(name="grep", bufs=2))
        dpool = ctx.enter_context(tc.tile_pool(name="dpool", bufs=1))
        work = ctx.enter_context(tc.tile_pool(name="work", bufs=1))
        small = ctx.enter_context(tc.tile_pool(name="small", bufs=2))

        for _rep in range(reps):
            # ---- one DMA for all dense inputs ----
            big = inp.tile([P, _IN_W], f16, tag="in")
            nc.sync.dma_start(big[:], INd[:])
            x16 = big[:, _OX:_OX + RT * D].rearrange("p (k d) -> p k d", k=RT)
            xt16 = big[:, _OXT:_OXT + NT * R].rearrange(
                "p (t n) -> p t n", t=NT)
            g16 = big[:, _OG:_OG + NT * C].rearrange("p (t c) -> p t c", t=NT)
            gt16 = big[0:C, _OGT:_OGT + D]
            ident = big[:, _OID:_OID + 2 * P].bitcast(f32)

            # ---- norms: rxn = 1/|x| [128, 4]; rgn = 1/|g| [100, 1] ----
            # (sqrt+reciprocal for both packed into single [128, 5] ops)
            xsq = dpool.tile([P, RT, D], f32, tag="diff", name="xsq")
            nc.vector.tensor_tensor(xsq[:], x16, x16, Alu.mult)
            nrm2 = small.tile([P, RT + 1], f32, tag="nrm2")
            nc.vector.tensor_reduce(nrm2[:, 0:RT], xsq[:], Ax.X, Alu.add)
            junk = dpool.tile([C, D], f32, tag="diff", name="junk")
            nc.scalar.activation(junk[:], gt16, Act.Square,
                                 accum_out=nrm2[0:C, RT:RT + 1])
            nrm = small.tile([P, RT + 1], f32, tag="nrm")
            nc.scalar.activation(nrm[:], nrm2[:], Act.Sqrt)
            rall = small.tile([P, RT + 1], f32, tag="rall")
            nc.vector.reciprocal(rall[:], nrm[:])
            rxn = rall[:, 0:RT]
            rgn = rall[0:C, RT:RT + 1]

            # ---- cosine GEMM, transposed: dotT[c, n] ----
            with tc.tile_pool(name="ps", bufs=2, space="PSUM") as psp:
                dotT = psp.tile([C, R], f32, tag="dotT")
                for t in range(NT):
                    nc.tensor.matmul(
                        dotT[:], lhsT=g16[:, t, :], rhs=xt16[:, t, :],
                        start=(t == 0), stop=(t == NT - 1))
                csT = work.tile([C, R], f32, tag="csT")
                nc.vector.tensor_tensor(
                    csT[:], dotT[:], rgn.broadcast_to([C, R]), Alu.mult)
                # transpose back to [n, c] layout: cst [128, RT, C] (PSUM)
                cst = psp.tile([P, RT, C], f32, tag="cst")
                for k in range(RT):
                    nc.tensor.transpose(
                        cst[:, k, :], csT[:, k * P:(k + 1) * P],
                        ident[0:C, 0:C])

                # ---- L1 distances: l1[n-part, k, c] ----
                l1 = work.tile([P, RT, C], f32, tag="l1")
                for cb in range(NCB):
                    grepb = grep_pool.tile([P, CB * D], f16, tag="grep")
                    nc.sync.dma_start(
                        grepb[:],
                        GR16d[0:1, cb * CB * D:(cb + 1) * CB * D]
                        .partition_broadcast(P))
                    diff = dpool.tile([P, RT, CB, D], f16, tag="diff")
                    nc.vector.tensor_tensor(
                        diff[:],
                        x16.unsqueeze(2).broadcast_to([P, RT, CB, D]),
                        grepb[:].rearrange("p (c d) -> p c d", c=CB)
                        .unsqueeze(1).broadcast_to([P, RT, CB, D]),
                        Alu.subtract)
                    nc.vector.tensor_reduce(
                        l1[:, :, cb * CB:(cb + 1) * CB], diff[:], Ax.X,
                        Alu.add, apply_absolute_value=True)

                # ---- epilogue, fused over all row-tiles ----
                cs = work.tile([P, RT, C], f32, tag="cs")
                nc.vector.tensor_tensor(
                    cs[:], cst[:],
                    rxn.unsqueeze(2).broadcast_to([P, RT, C]), Alu.mult)
            conf = small.tile([P, RT], f32, tag="conf")
            nc.vector.tensor_reduce(conf[:], cs[:], Ax.X, Alu.max)

            # e2 = exp(cs) (|cs|<=1, no shift); e1 = exp(-(l1 - m2)) with a
            # per-partition min shift (exact: softmax ratios are invariant
            # to per-row constants and rows live on single partitions)
            e12 = work.tile([P, 2 * RT, C], f32, tag="e12")
            nc.scalar.activation(e12[:, 0:RT, :], cs[:], Act.Exp)
            m2 = small.tile([P, 1], f32, tag="m2")
            nc.vector.tensor_reduce(m2[:], l1[:], Ax.XY, Alu.min)
            nc.scalar.activation(e12[:, RT:2 * RT, :], l1[:], Act.Exp,
                                 bias=m2[:], scale=-1.0)
            s12 = small.tile([P, 2 * RT], f32, tag="s12")
            nc.vector.tensor_reduce(s12[:], e12[:], Ax.X, Alu.add)

            # out = (e1*e2) * (conf / (s1*s2))
            den = small.tile([P, RT], f32, tag="den")
            nc.vector.tensor_tensor(
                den[:], s12[:, RT:2 * RT], s12[:, 0:RT], Alu.mult)
            rden = small.tile([P, RT], f32, tag="rden")
            nc.vector.reciprocal(rden[:], den[:])
            fac = small.tile([P, RT], f32, tag="fac")
            nc.vector.tensor_tensor(fac[:], conf[:], rden[:], Alu.mult)
            prod = work.tile([P, RT, C], f32, tag="prod")
            nc.vector.tensor_tensor(
                prod[:], e12[:, RT:2 * RT, :], e12[:, 0:RT, :], Alu.mult)
            outt = work.tile([P, RT, C], f32, tag="outt")
            nc.vector.tensor_tensor(
                outt[:], prod[:],
                fac[:].unsqueeze(2).broadcast_to([P, RT, C]), Alu.mult)
            nc.sync.dma_start(
                Yd[:].rearrange("(k p) c -> p k c", p=P), outt[:])

    _split_excess_waits(nc)
    return nc


def prep_in_maps(X: np.ndarray, grp: np.ndarray):
    """Host-side sharding + layout prep (per the data-parallel hint)."""
    X16 = np.ascontiguousarray(X, dtype=np.float32).astype(np.float16)
    g = np.ascontiguousarray(grp.reshape(D, C), dtype=np.float32)
    G16 = g.astype(np.float16)
    GT16 = np.ascontiguousarray(G16.T)                    # [100, 512]
    GR16 = GT16.reshape(1, C * D)                         # c-major flat
    ident = np.eye(P, dtype=np.float32).view(np.float16)  # [128, 256]

    in_maps = []
    for s in range(N_CORES):
        xs = X16[s * R:(s + 1) * R]
        IN = np.zeros((P, _IN_W), dtype=np.float16)
        IN[:, _OX:_OX + RT * D] = (
            xs.reshape(RT, P, D).transpose(1, 0, 2).reshape(P, RT * D))
        IN[:, _OXT:_OXT + NT * R] = (
            np.ascontiguousarray(xs.T).reshape(NT, P, R)
            .transpose(1, 0, 2).reshape(P, NT * R))
        IN[:, _OG:_OG + NT * C] = (
            G16.reshape(NT, P, C).transpose(1, 0, 2).reshape(P, NT * C))
        IN[0:C, _OGT:_OGT + D] = GT16
        IN[:, _OID:_OID + 2 * P] = ident
        in_maps.append({"IN16": IN, "GR16": GR16})
    return in_maps


def kernel(X: np.ndarray, grp: np.ndarray) -> np.ndarray:
    from concourse.bass_utils import run_bass_kernel_spmd

    if "nc" not in _CACHE:
        _CACHE["nc"] = _build_nc()
    nc = _CACHE["nc"]

    in_maps = prep_in_maps(X, grp)
    last_err = None
    for _attempt in range(3):
        try:
            res = run_bass_kernel_spmd(nc, in_maps, list(range(N_CORES)))
            break
        except Exception as e:  # transient device/tunnel hiccups
            last_err = e
            import time
            time.sleep(2.0)
    else:
        raise last_err
    out = np.concatenate(
        [res.results[i]["Y"] for i in range(N_CORES)], axis=0
    )
    return np.ascontiguousarray(out, dtype=np.float32)
